# revision 21
# baseline (speedup 1.0000x reference)
"""Distributed GQA attention-with-cache kernel for 8 TRN2 NeuronCores.

Tensor-parallel over heads: core c owns q-heads [4c, 4c+4) and kv-head c.

v3 design: scores are computed TRANSPOSED (K-chunk stationary, q moving) so
the exp'd probabilities land directly in the [t, q] layout the
attention-times-V matmul wants as its stationary operand — no P-assembly
DMAs and no P transposes.  AV runs 512-wide ping-ponging two PSUM banks
(the measured-fast shape); softmax denominators come from near-free N=1
matmuls against a ones column.  The V cache streams in fp8-e3m4 (half the
HBM bytes, ~1.1% output error), K stays bf16.  All DRAM operands are
pre-arranged host-side into [128, free] partition-major layouts so every
DMA stream moves 4-8KB contiguous runs per partition (equal descriptor
sizes keep the SDMA round-robin from starving any stream).  Per-core
partial output projections are summed on the host (no on-device
collective).
"""
import numpy as np
import ml_dtypes

import concourse.bass as bass  # noqa: F401
import concourse.mybir as mybir
import concourse.tile as tile
from concourse import bacc
from concourse.bass_utils import run_bass_kernel_spmd
from concourse.masks import make_identity

# If BASS_TRACE is set but the axon NTFF hook module is absent, bass_utils
# would fail on import; provide a no-op stub so tracing degrades gracefully.
try:
    import antenv.axon_hooks  # noqa: F401
except Exception:
    import sys as _sys
    import types as _types

    _m = _types.ModuleType("antenv.axon_hooks")
    _m.get_axon_ntff_profile_hook = lambda: None
    _m.set_axon_ntff_profile_hook = lambda h: None
    _sys.modules["antenv.axon_hooks"] = _m

B, S, T, L, NH, NKV, HD, DIM = 8, 4, 2048, 2, 32, 8, 128, 4096
N_CORES = 8
HPC = NH // N_CORES          # 4 q-heads per core
CW = HPC * HD                # 512 attn feature cols per core
NTOK = B * S                 # 32 tokens
QKVW = CW + 2 * HD           # 768: q(512) | k(128) | v(128)
RPB = HPC * S                # 16 q-rows per batch: (h, s)
NCH = T // 128               # 16 t-chunks per batch

F32 = mybir.dt.float32
BF16 = mybir.dt.bfloat16
FP8 = mybir.dt.float8e3
AF = mybir.ActivationFunctionType
ALU = mybir.AluOpType

_CACHE = {}


def _build():
    ndc = DIM // 128         # 32 contraction chunks for the projections

    nc = bacc.Bacc("TRN2", target_bir_lowering=False, debug=False, num_devices=N_CORES)
    # all layouts pre-arranged host-side to [partition=128, free] contiguous
    xT = nc.declare_dram_parameter("xT", [128, ndc * NTOK], BF16, isOutput=False)
    wqkvT = nc.declare_dram_parameter("wqkvT", [8, 128, 4 * QKVW], BF16, isOutput=False)
    woT = nc.declare_dram_parameter("woT", [128, HPC * DIM], BF16, isOutput=False)
    kT = nc.declare_dram_parameter("kT", [B, 128, HPC * T], BF16, isOutput=False)
    vC8 = nc.declare_dram_parameter("vC8", [B, 128, NCH * CW], FP8, isOutput=False)
    ropes = nc.declare_dram_parameter("ropes", [NTOK, 4 * (HD // 2)], F32, isOutput=False)
    # 0/1 multiplier on the exp'd scoresT tile [t%128, (b, ch, q)] killing
    # the replaced cache rows (identical over the 16 q columns).
    maskT = nc.declare_dram_parameter("maskT", [128, B * NCH * RPB], BF16, isOutput=False)
    out = nc.declare_dram_parameter("out", [NTOK, DIM], BF16, isOutput=True)

    with tile.TileContext(nc) as tc:
        with (
            tc.tile_pool(name="const", bufs=1) as cn,
            tc.tile_pool(name="kpool", bufs=3) as kp,
            tc.tile_pool(name="vpool", bufs=4) as vp,
            tc.tile_pool(name="stg", bufs=2) as st,
            tc.tile_pool(name="ppool", bufs=3) as pp,
            tc.tile_pool(name="wqkvp", bufs=8) as wqp,
        ):
            # x first (the projection's stationary operand), then the wqkv
            # pieces across ALL THREE queues (full HBM bandwidth — pipeline
            # start hinges on the weights), then K/V/masks behind them.
            xT_sb = cn.tile([128, ndc * NTOK], BF16)
            nc.sync.dma_start(xT_sb[:], xT[:])

            wqp_t = []
            wengs = [nc.sync, nc.scalar, nc.gpsimd]
            for pc in range(8):
                wt = wqp.tile([128, 4 * QKVW], BF16, tag="wqkv", name=f"wp{pc}")
                wengs[pc % 3].dma_start(wt[:], wqkvT[pc, :, :])
                wqp_t.append(wt)

            ktb_t = [kp.tile([128, HPC * T], BF16, tag="kt", name=f"ktb{b}")
                     for b in range(B)]
            vtb_t = [vp.tile([128, NCH * CW], FP8, tag="v", name=f"vtb{b}")
                     for b in range(B)]

            def load_k(b, engs=(nc.sync, nc.sync)):
                for half in range(2):
                    engs[half].dma_start(
                        ktb_t[b][:, half * 2 * T:(half + 1) * 2 * T],
                        kT[b, :, half * 2 * T:(half + 1) * 2 * T])

            def load_v(b):
                for half in range(2):
                    nc.gpsimd.dma_start(
                        vtb_t[b][:, half * 8 * CW:(half + 1) * 8 * CW],
                        vC8[b, :, half * 8 * CW:(half + 1) * 8 * CW])

            rope_sb = cn.tile([NTOK, 4 * (HD // 2)], F32)
            nc.scalar.dma_start(rope_sb[:], ropes[:])
            # K0/K1 halves spread across queues: right after the weight
            # pieces drain, the first K batches ride full HBM bandwidth.
            load_k(0, engs=(nc.sync, nc.scalar))
            load_k(1, engs=(nc.sync, nc.gpsimd))
            mask_sb = cn.tile([128, B * NCH * RPB], BF16)
            nc.sync.dma_start(mask_sb[:], maskT[:])

            ident = cn.tile([128, 128], F32)
            make_identity(nc, ident[:])
            identB = cn.tile([RPB, RPB], BF16)
            nc.vector.tensor_copy(identB[:], ident[:RPB, :RPB])

            # PE clock warmup: ~4.5us of junk transposes so the HAM gate is
            # at 8/8 (2.4 GHz) by the time the first weight piece lands.
            # Also preload the scalar engine's EXP table (1.5us one-time)
            # off the critical path.
            junkE = cn.tile([RPB, 1], F32)
            with tc.tile_pool(name="psW", bufs=1, space="PSUM") as psW:
                warm = psW.tile([128, 128], F32, space="PSUM")
                for _ in range(30):
                    nc.tensor.transpose(warm[:], ident[:], ident[:])
            nc.scalar.activation(junkE[:], ident[:RPB, 0:1], AF.Exp)

            qkv_sb = cn.tile([NTOK, QKVW], F32)
            qrot = cn.tile([NTOK, CW], F32)
            krot = cn.tile([NTOK, HD], F32)
            qT_sb = cn.tile([128, NTOK * HPC], BF16)   # cols = (b, h, s)
            knT_sb = cn.tile([128, NTOK], BF16)        # cols = (b, s)
            vkv = cn.tile([S, B * HD], BF16)           # new v rows s', cols (b, d)
            ones2 = cn.tile([S, 1], BF16)
            onesP = cn.tile([128, 1], BF16)
            nc.vector.memset(ones2[:], 1.0)
            nc.vector.memset(onesP[:], 1.0)

            # ---------------- phase A: projections + RoPE -----------------
            if True:
                with tc.tile_pool(name="psP", bufs=1, space="PSUM") as psP:
                    qkv_ps = psP.tile([NTOK, QKVW], F32, space="PSUM")
                    npc = ndc // 8  # 4 chunks per piece
                    for pc in range(8):
                        wt = wqp_t[pc]
                        for cc in range(npc):
                            c = pc * npc + cc
                            lhs = xT_sb[:, c * NTOK:(c + 1) * NTOK]
                            rr = wt[:, cc * QKVW:(cc + 1) * QKVW]
                            nc.tensor.matmul(qkv_ps[:, 0:512], lhs, rr[:, 0:512],
                                             start=(c == 0), stop=(c == ndc - 1))
                            nc.tensor.matmul(qkv_ps[:, 512:QKVW], lhs, rr[:, 512:QKVW],
                                             start=(c == 0), stop=(c == ndc - 1))
                    nc.vector.tensor_copy(qkv_sb[:], qkv_ps[:])

                    # RoPE (q scaled by 1/sqrt(HD) via cq/sq; k unscaled)
                    HH = HD // 2
                    cq, sq = rope_sb[:, 0:HH], rope_sb[:, HH:2 * HH]
                    ck, sk = rope_sb[:, 2 * HH:3 * HH], rope_sb[:, 3 * HH:4 * HH]
                    t1 = cn.tile([NTOK, HH], F32)
                    t2 = cn.tile([NTOK, HH], F32)

                    def rope(src_ap, dst_ap, c_t, s_t):
                        sv = src_ap.rearrange("p (i two) -> p two i", two=2)
                        dv = dst_ap.rearrange("p (i two) -> p two i", two=2)
                        nc.vector.tensor_tensor(t1[:], sv[:, 0, :], c_t, op=ALU.mult)
                        nc.vector.tensor_tensor(t2[:], sv[:, 1, :], s_t, op=ALU.mult)
                        nc.vector.tensor_tensor(dv[:, 0, :], t1[:], t2[:], op=ALU.subtract)
                        nc.vector.tensor_tensor(t1[:], sv[:, 0, :], s_t, op=ALU.mult)
                        nc.vector.tensor_tensor(t2[:], sv[:, 1, :], c_t, op=ALU.mult)
                        nc.vector.tensor_tensor(dv[:, 1, :], t1[:], t2[:], op=ALU.add)

                    for h in range(HPC):
                        rope(qkv_sb[:, h * HD:(h + 1) * HD], qrot[:, h * HD:(h + 1) * HD], cq, sq)
                    rope(qkv_sb[:, CW:CW + HD], krot[:], ck, sk)

                    # transposes: qT cols (b, h, s); k_new^T cols (b, s)
                    for h in range(HPC):
                        tp = psP.tile([128, NTOK], F32, tag="tp", space="PSUM")
                        nc.tensor.transpose(tp[:], qrot[:, h * HD:(h + 1) * HD], ident[:NTOK, :NTOK])
                        nc.vector.tensor_copy(
                            qT_sb[:].rearrange("p (b h s) -> p b h s", h=HPC, s=S)[:, :, h, :],
                            tp[:].rearrange("p (b s) -> p b s", s=S),
                        )
                    tp = psP.tile([128, NTOK], F32, tag="tp", space="PSUM")
                    nc.tensor.transpose(tp[:], krot[:], ident[:NTOK, :NTOK])
                    nc.vector.tensor_copy(knT_sb[:], tp[:])

                    # new v rows: partition-compact tokens of batch b to rows
                    # 0:4 (the new-position AV matmul's moving operand)
                    vkstg = cn.tile([S, B * HD], F32)
                    for b in range(B):
                        nc.sync.dma_start(vkstg[:, b * HD:(b + 1) * HD],
                                          qkv_sb[b * S:(b + 1) * S, CW + HD:QKVW])
                    nc.vector.tensor_copy(vkv[:], vkstg[:])

            wo_t = cn.tile([128, HPC * DIM], BF16)

            # ---- per-batch pipeline: scoresT -> exp -> mask -> AV --------
            rec = cn.tile([RPB, B], F32)   # col b = 1/den for batch b
            attnT = cn.tile([128, HPC * NTOK], BF16)  # cols (h, tok)

            with (
                tc.tile_pool(name="psS", bufs=2, space="PSUM") as psS,
                tc.tile_pool(name="psA", bufs=1, space="PSUM") as psA,
                tc.tile_pool(name="psD", bufs=1, space="PSUM") as psD,
                tc.tile_pool(name="psT", bufs=1, space="PSUM") as psT,
            ):
                P_t = [None] * B

                def emit_scores(b):
                    """K-stationary scoresT + newpos scores + exp + mask."""
                    scT = psS.tile([128, NCH * RPB + RPB], F32, tag="scT",
                                   space="PSUM", name=f"scT{b}")
                    for ch in range(NCH):
                        for h in range(HPC):
                            nc.tensor.matmul(
                                scT[:, ch * RPB + h * S: ch * RPB + (h + 1) * S],
                                ktb_t[b][:, h * T + ch * 128: h * T + (ch + 1) * 128],
                                qT_sb[:, b * RPB + h * S: b * RPB + (h + 1) * S],
                                start=True, stop=True,
                            )
                    # new-position scoresT block [s'=4, q=16] (4 N=4 matmuls
                    # to stay in the same moving-width class)
                    for h in range(HPC):
                        nc.tensor.matmul(scT[0:S, NCH * RPB + h * S:NCH * RPB + (h + 1) * S],
                                         knT_sb[:, b * S:(b + 1) * S],
                                         qT_sb[:, b * RPB + h * S:b * RPB + (h + 1) * S],
                                         start=True, stop=True)
                    P_b = pp.tile([128, NCH * RPB + RPB], BF16, tag="P", name=f"P{b}")
                    P_t[b] = P_b
                    nc.scalar.activation(P_b[:], scT[:], AF.Exp)
                    # kill the replaced cache rows
                    nc.vector.tensor_tensor(
                        P_b[:, 0:NCH * RPB], P_b[:, 0:NCH * RPB],
                        mask_sb[:, b * NCH * RPB:(b + 1) * NCH * RPB],
                        op=ALU.mult)

                def emit_av(b):
                    """den (N=1 matmuls) + 512-wide ping-pong AV + newpos,
                    normalize, transpose-gather."""
                    P_b = P_t[b]
                    pnew = P_b[0:S, NCH * RPB:NCH * RPB + RPB]
                    vnb = vkv[:, b * HD:(b + 1) * HD]

                    den_e = psD.tile([RPB, 1], F32, tag="de", space="PSUM",
                                     name=f"de{b}")
                    den_o = psD.tile([RPB, 1], F32, tag="do", space="PSUM",
                                     name=f"do{b}")
                    for ch in range(NCH):
                        t = den_e if ch % 2 == 0 else den_o
                        nc.tensor.matmul(t[:], P_b[:, ch * RPB:(ch + 1) * RPB],
                                         onesP[:], start=(ch < 2),
                                         stop=(ch == NCH - 1))
                    nc.tensor.matmul(den_e[:], pnew, ones2[:], start=False, stop=True)

                    av_e = psA.tile([RPB, CW], F32, tag="ave", space="PSUM",
                                    name=f"ave{b}")
                    av_o = psA.tile([RPB, CW], F32, tag="avo", space="PSUM",
                                    name=f"avo{b}")
                    for ch in range(NCH):
                        t = av_e if ch % 2 == 0 else av_o
                        nc.tensor.matmul(t[:], P_b[:, ch * RPB:(ch + 1) * RPB],
                                         vtb_t[b][:, ch * CW:(ch + 1) * CW],
                                         start=(ch < 2),
                                         stop=(ch == NCH - 2))
                    # new-position contributions (v unrepeated; per head)
                    for h in range(HPC):
                        nc.tensor.matmul(av_o[:, h * HD:(h + 1) * HD], pnew, vnb,
                                         start=False, stop=True)
                    if b + 4 < B:
                        load_k(b + 4)
                        load_v(b + 4)

                    # rec = 1/(den_e + den_o); av = (av_e + av_o) * rec
                    rc = rec[:, b:b + 1]
                    nc.vector.tensor_copy(rc, den_e[:])
                    nc.vector.tensor_tensor(rc, rc, den_o[:], op=ALU.add)
                    nc.vector.reciprocal(rc, rc)
                    av_sb = st.tile([RPB, CW], F32, tag="avsb")
                    nc.vector.tensor_copy(av_sb[:], av_e[:])
                    nc.vector.tensor_tensor(av_sb[:], av_sb[:], av_o[:], op=ALU.add)
                    av_sc = st.tile([RPB, CW], BF16, tag="avsc")
                    nc.vector.tensor_scalar_mul(av_sc[:], av_sb[:], rc)
                    # transpose per head; keep the 4 valid q columns
                    tpx = psT.tile([128, HPC * RPB], BF16, tag="tp4", space="PSUM")
                    for h in range(HPC):
                        nc.tensor.transpose(tpx[:, h * RPB:(h + 1) * RPB],
                                            av_sc[:, h * HD:(h + 1) * HD],
                                            identB[:])
                    for h in range(HPC):
                        nc.vector.tensor_copy(
                            attnT[:, h * NTOK + b * S: h * NTOK + (b + 1) * S],
                            tpx[:, h * RPB + h * S: h * RPB + (h + 1) * S],
                        )

                # 1-batch software-pipeline skew
                for i in range(4):
                    nc.scalar.dma_start(wo_t[:, i * HPC * 1024:(i + 1) * HPC * 1024],
                                        woT[:, i * HPC * 1024:(i + 1) * HPC * 1024])
                load_v(0)
                load_v(1)
                load_k(2)
                load_v(2)
                load_k(3)
                load_v(3)
                emit_scores(0)
                for b in range(1, B):
                    emit_scores(b)
                    emit_av(b - 1)
                emit_av(B - 1)

            # ------ phase G: PARTIAL output projection (no collective) ----
            with tc.tile_pool(name="psY", bufs=2, space="PSUM") as psY:
                for oc in range(DIM // 512):
                    y_ps = psY.tile([NTOK, 512], F32, tag="yps", space="PSUM")
                    for c in range(HPC):
                        nc.tensor.matmul(
                            y_ps[:],
                            attnT[:, c * NTOK:(c + 1) * NTOK],
                            wo_t[:, c * DIM + oc * 512:c * DIM + (oc + 1) * 512],
                            start=(c == 0), stop=(c == HPC - 1),
                        )
                    y_sb = st.tile([NTOK, 512], BF16, tag="ysb")
                    nc.vector.tensor_copy(y_sb[:], y_ps[:])
                    nc.sync.dma_start(out[:, oc * 512:(oc + 1) * 512], y_sb[:])

    nc.compile()
    return nc


def _get_nc():
    if "nc" not in _CACHE:
        _CACHE["nc"] = _build()
    return _CACHE["nc"]


def _bf16(a):
    return np.ascontiguousarray(a).astype(ml_dtypes.bfloat16)


def _part_major(a):
    """[C*128, F] -> [128, C*F] partition-major relayout."""
    c128, f = a.shape
    c = c128 // 128
    return np.ascontiguousarray(
        a.reshape(c, 128, f).transpose(1, 0, 2).reshape(128, c * f))


def _prep_in_maps(x, start_pos, angles, cache_k, cache_v, wq, wk, wv, wo, layer_idx):
    li = int(layer_idx)
    xf = np.asarray(x, np.float32).reshape(NTOK, DIM).T        # [DIM, 32]
    ang = np.asarray(angles, np.float64).reshape(NTOK, HD // 2)
    alpha = 1.0 / np.sqrt(HD)
    ropes = np.concatenate([np.cos(ang) * alpha, np.sin(ang) * alpha,
                            np.cos(ang), np.sin(ang)], axis=1).astype(np.float32)
    sp = np.asarray(start_pos).astype(np.int64)

    # scoresT mask: [p, (b, ch, q)] = 0 where global t = ch*128+p is one of
    # the replaced cache rows [sp_b, sp_b+S), else 1 (identical over q).
    maskT = np.ones((128, B, NCH, RPB), np.float32)
    for b in range(B):
        for t in range(sp[b], sp[b] + S):
            maskT[t % 128, b, t // 128, :] = 0.0
    maskT = _bf16(maskT.reshape(128, B * NCH * RPB))

    wq = np.asarray(wq, np.float32)
    wk = np.asarray(wk, np.float32)
    wv = np.asarray(wv, np.float32)
    wo = np.asarray(wo, np.float32)
    ck_l = np.asarray(cache_k, np.float32)[:, :, li, :]
    cv_l = np.asarray(cache_v, np.float32)[:, :, li, :]

    in_maps = []
    for c in range(N_CORES):
        qs, qe = c * CW, (c + 1) * CW
        ks, ke = c * HD, (c + 1) * HD
        # [DIM, QKVW] -> pieces [8, 128, 4*QKVW] partition-major
        wqkvT = np.concatenate([wq[qs:qe].T, wk[ks:ke].T, wv[ks:ke].T], axis=1)
        wqkvT = _part_major(wqkvT).reshape(128, 8, 4 * QKVW).transpose(1, 0, 2)
        # K: [T, CW] -> [CW, T] -> [128, (h, T)] partition-major
        kTc = _part_major(ck_l[:, :, qs:qe].transpose(0, 2, 1).reshape(B * CW, T)
                          .reshape(B * CW, T)).reshape(128, B, HPC * T)
        kTc = np.ascontiguousarray(kTc.transpose(1, 0, 2))
        # V: [T, CW] -> [128, (ch, CW)] partition-major chunks, fp8-e3m4
        v8 = np.clip(cv_l[:, :, qs:qe], -15.5, 15.5).astype(ml_dtypes.float8_e3m4)
        v8 = v8.reshape(B, NCH, 128, CW).transpose(0, 2, 1, 3).reshape(B, 128, NCH * CW)
        in_maps.append({
            "xT": _bf16(_part_major(xf)),
            "wqkvT": _bf16(np.ascontiguousarray(wqkvT)),
            "woT": _bf16(_part_major(wo[:, qs:qe].T)),
            "kT": _bf16(kTc),
            "vC8": np.ascontiguousarray(v8),
            "ropes": ropes,
            "maskT": maskT,
        })
    return in_maps


def kernel(x, start_pos, angles, cache_k, cache_v, mask, wq, wk, wv, wo, layer_idx):
    del mask  # zeros by construction
    in_maps = _prep_in_maps(x, start_pos, angles, cache_k, cache_v, wq, wk, wv, wo, layer_idx)
    nc = _get_nc()
    res = run_bass_kernel_spmd(nc, in_maps, core_ids=list(range(N_CORES)))
    _CACHE["last_result"] = res
    y = np.sum([np.asarray(res.results[c]["out"], np.float32)
                for c in range(N_CORES)], axis=0)
    return y.reshape(B, S, DIM)
